# revision 19
# baseline (speedup 1.0000x reference)
"""GNN edge-MLP kernel for 8 TRN2 NeuronCores.

reference:
    xs = x1[edge_index[0]]; xt = x2[edge_index[1]]
    h = relu(concat(xs, xt) @ W1 + b1); h = relu(h @ W2 + b2); out = h @ W3 + b3

Strategy (pure edge parallelism, no collectives):
  - Edges sharded 8 ways (200k per core); node tables + weights replicated.
  - Host-side prep (numpy): cast tables/weights to bf16, bucket each core's
    edges by (src_chunk, dst_chunk) over 4 node chunks of 25000 rows so local
    gather indices fit in int16, pad buckets to a fixed capacity (pad idx=-1),
    pre-wrap indices into dma_gather's [16 x n/16] interleaved layout.
  - Device kernel: NON-transposed dma_gather (SWDGE) striped over all 4 SWDGE
    queues pulls edge rows row-major [128 edges x 128 feat] into SBUF.
    (Transposed gathers serialize on the shared spray xbar, so they cannot
    scale past ~1 ring; non-transposed gathers scale with the 4 rings.)
    The TensorEngine transposes each 128-edge block via identity matmul into
    PSUM (bf16), ACT/DVE copy the transposed tiles to SBUF, then the MLP:
      L1: psum = W1a.T @ xsT + W1b.T @ xtT     (two accumulating matmuls)
      s1 = relu(psum + b1)                      (DVE tensor_scalar add+max)
      L2: psum2 = W2.T @ s1 ; s2 = relu(psum2+b2)  (ACT activation)
      L3: obank[row] = W3.T @ s2                (M=1 matmul into a shared
                                                 psum bank, one row per tile)
    Output rows are flushed 32 tiles at a time (+ b3) to DRAM.
  - Host unpermutes bucket order back to the original edge order.
"""

import sys

sys.path.insert(0, "/opt/trn_rl_repo")

import functools

import ml_dtypes
import numpy as np

import concourse.bacc as bacc
import concourse.bass as bass
import concourse.mybir as mybir
import concourse.tile as tile
from concourse import library_config
from concourse.bass_utils import run_bass_kernel_spmd

P = 128
D = 128
N_NODES = 100000
N_EDGES = 1600000
N_CORES = 8
E_CORE = N_EDGES // N_CORES  # 200000
NCHUNK = 4
CHUNK = N_NODES // NCHUNK  # 25000 rows per node chunk (int16-safe)
NBUCKET = NCHUNK * NCHUNK  # 16
MM_N = 512  # matmul moving free dim (one PSUM f32 bank)
CAP = 13312  # per-bucket padded edge capacity (26 * 512); mean fill is 12500
TCALL = 3584  # idxs per bucket gathered TRANSPOSED (xbar path, serialized)
NTCALL = 3584  # max idxs per non-transposed gather call
FLUSH = 32  # tiles whose [1, 512] outputs accumulate in one psum bank

BF16 = mybir.dt.bfloat16
F32 = mybir.dt.float32
I16 = mybir.dt.int16
nbf = ml_dtypes.bfloat16


@functools.lru_cache(maxsize=2)
def build(cap=CAP):
    assert cap % MM_N == 0
    tcall = TCALL if cap >= 2 * TCALL else (cap // 2 // MM_N) * MM_N
    ntc = cap - tcall  # non-transposed idxs per bucket
    # 3 NT calls; sizes chosen so the 8 SWDGE instrs per bucket, assigned
    # queue = instr_index % 4, put equal descriptor volume on every ring.
    a = min(NTCALL, ntc - 2 * ((ntc // 3) // MM_N * MM_N))
    rem = ntc - a
    nt_sizes = [a, rem // 2, rem - rem // 2]
    assert all(sz > 0 and sz % MM_N == 0 for sz in nt_sizes), nt_sizes
    assert len(nt_sizes) == 3
    ntiles = NBUCKET * cap // MM_N
    nc = bacc.Bacc("TRN2", num_swdge_queues=4)
    x1bf = nc.dram_tensor("x1bf", [N_NODES, D], BF16, kind="ExternalInput")
    x2bf = nc.dram_tensor("x2bf", [N_NODES, D], BF16, kind="ExternalInput")
    sidx = nc.dram_tensor("sidx", [NBUCKET, P, cap // 16], I16, kind="ExternalInput")
    didx = nc.dram_tensor("didx", [NBUCKET, P, cap // 16], I16, kind="ExternalInput")
    w1a = nc.dram_tensor("w1a", [D, D], BF16, kind="ExternalInput")
    w1b = nc.dram_tensor("w1b", [D, D], BF16, kind="ExternalInput")
    w2 = nc.dram_tensor("w2", [D, D], BF16, kind="ExternalInput")
    w3 = nc.dram_tensor("w3", [D, 1], BF16, kind="ExternalInput")
    b1 = nc.dram_tensor("b1", [D, 1], F32, kind="ExternalInput")
    b2 = nc.dram_tensor("b2", [D, 1], F32, kind="ExternalInput")
    b3 = nc.dram_tensor("b3", [P, 1], F32, kind="ExternalInput")
    ident = nc.dram_tensor("ident", [P, P], BF16, kind="ExternalInput")
    out = nc.dram_tensor("out", [ntiles, MM_N], F32, kind="ExternalOutput")

    Relu = mybir.ActivationFunctionType.Relu
    Add = mybir.AluOpType.add
    Max = mybir.AluOpType.max

    with tile.TileContext(nc) as tc:
        nc.gpsimd.load_library(library_config.mlp)
        with (
            tc.tile_pool(name="const", bufs=1) as cpool,
            tc.tile_pool(name="gxs", bufs=2) as gxs_pool,
            tc.tile_pool(name="gxt", bufs=2) as gxt_pool,
            tc.tile_pool(name="txs", bufs=2) as txs_pool,
            tc.tile_pool(name="txt", bufs=2) as txt_pool,
            tc.tile_pool(name="idx", bufs=4) as idx_pool,
            tc.tile_pool(name="tidx", bufs=1) as tidx_pool,
            tc.tile_pool(name="tsb", bufs=4) as tsb_pool,
            tc.tile_pool(name="act", bufs=3) as act_pool,
            tc.tile_pool(name="osb", bufs=2) as out_pool,
            tc.tile_pool(name="ptr", bufs=2, space="PSUM") as ptr_pool,
            tc.tile_pool(name="ph", bufs=2, space="PSUM") as ph_pool,
            tc.tile_pool(name="pob", bufs=2, space="PSUM") as pob,
        ):
            w1a_sb = cpool.tile([D, D], BF16, tag="w1a")
            w1b_sb = cpool.tile([D, D], BF16, tag="w1b")
            w2_sb = cpool.tile([D, D], BF16, tag="w2")
            w3_sb = cpool.tile([D, 1], BF16, tag="w3")
            b1_sb = cpool.tile([D, 1], F32, tag="b1")
            b2_sb = cpool.tile([D, 1], F32, tag="b2")
            b3_sb = cpool.tile([P, 1], F32, tag="b3")
            id_sb = cpool.tile([P, P], BF16, tag="ident")
            for sb, dr in [
                (w1a_sb, w1a), (w1b_sb, w1b), (w2_sb, w2), (w3_sb, w3),
                (b1_sb, b1), (b2_sb, b2), (b3_sb, b3), (id_sb, ident),
            ]:
                nc.sync.dma_start(sb[:], dr[:])

            # w3 shifted into column m of slice m: L3's matmul for the m-th
            # tile of a flush group lands its [1, 512] result on psum
            # partition m (accumulating zeros onto every other row).
            w3m_sb = cpool.tile([P, FLUSH, D], BF16, tag="w3m")
            nc.vector.memset(w3m_sb[:], 0.0)
            for m in range(FLUSH):
                nc.vector.tensor_copy(w3m_sb[:, m, m : m + 1], w3_sb[:, 0:1])

            obank = None
            orow = 0
            oflushed = 0
            tile_no = 0

            def flush(rows):
                nonlocal obank, orow, oflushed
                osb = out_pool.tile([P, MM_N], F32, tag="osb")
                nc.vector.tensor_scalar_add(
                    osb[:rows, :], obank[:rows, :], b3_sb[:rows, 0:1]
                )
                nc.sync.dma_start(out[oflushed : oflushed + rows, :], osb[:rows, :])
                oflushed += rows
                obank = None
                orow = 0

            gi = 0  # SWDGE instruction counter; queue = gi % 4 always
            for b in range(NBUCKET):
                ci, cj = b // NCHUNK, b % NCHUNK
                x1c = x1bf[ci * CHUNK : (ci + 1) * CHUNK, :]
                x2c = x2bf[cj * CHUNK : (cj + 1) * CHUNK, :]

                pend = []  # buffered tile inputs, flushed in pairs

                def run_pair(pair):
                    nonlocal obank, orow, tile_no
                    # Same-stationary matmuls issued back-to-back so the PE
                    # keeps each weight matrix loaded for both tiles.
                    h1s = []
                    for xsT_ap, xtT_ap in pair:
                        h1 = ph_pool.tile([P, MM_N], F32, tag="h1")
                        h1s.append(h1)
                        nc.tensor.matmul(
                            h1[:], w1a_sb[:], xsT_ap, start=True, stop=False
                        )
                    for h1, (xsT_ap, xtT_ap) in zip(h1s, pair):
                        nc.tensor.matmul(
                            h1[:], w1b_sb[:], xtT_ap, start=False, stop=True
                        )
                    s1s = []
                    for h1 in h1s:
                        s1 = act_pool.tile([P, MM_N], BF16, tag="s1")
                        s1s.append(s1)
                        nc.vector.tensor_scalar(
                            s1[:], h1[:], b1_sb[:, 0:1], 0.0, Add, Max
                        )
                    h2s = []
                    for s1 in s1s:
                        h2 = ph_pool.tile([P, MM_N], F32, tag="h2")
                        h2s.append(h2)
                        nc.tensor.matmul(h2[:], w2_sb[:], s1[:], start=True, stop=True)
                    s2s = []
                    for h2 in h2s:
                        s2 = act_pool.tile([P, MM_N], BF16, tag="s2")
                        s2s.append(s2)
                        nc.scalar.activation(s2[:], h2[:], Relu, bias=b2_sb[:, 0:1])
                    for s2 in s2s:
                        if obank is None:
                            obank = pob.tile([P, MM_N], F32, tag="ob")
                        grp = min(FLUSH, ntiles - (tile_no - orow))
                        nc.tensor.matmul(
                            obank[:], w3m_sb[:, orow, :], s2[:],
                            start=(orow == 0), stop=(orow == grp - 1),
                        )
                        orow += 1
                        tile_no += 1
                        if orow == grp:
                            flush(grp)

                def run_tile(xsT_ap, xtT_ap, force=False):
                    pend.append((xsT_ap, xtT_ap))
                    if len(pend) == 2:
                        run_pair(pend[:])
                        pend.clear()

                def drain_tiles():
                    if pend:
                        run_pair(pend[:])
                        pend.clear()

                # --- non-transposed slices (rest of the bucket) ---
                base = 0
                for sz in nt_sizes:
                    nblk = sz // P
                    c0 = base // 16
                    cols = sz // 16
                    sidx_sb = idx_pool.tile([P, cols], I16, tag="sidx")
                    didx_sb = idx_pool.tile([P, cols], I16, tag="didx")
                    nc.sync.dma_start(sidx_sb[:], sidx[b, :, c0 : c0 + cols])
                    nc.sync.dma_start(didx_sb[:], didx[b, :, c0 : c0 + cols])
                    xs_rm = gxs_pool.tile([P, NTCALL // P, D], BF16, tag="xs")
                    xt_rm = gxt_pool.tile([P, NTCALL // P, D], BF16, tag="xt")
                    nc.gpsimd.dma_gather(
                        xs_rm[:, :nblk, :], x1c, sidx_sb[:], sz, sz, D,
                        transpose=False, single_packet=False,
                        queue_num=gi % 4,
                    )
                    nc.gpsimd.dma_gather(
                        xt_rm[:, :nblk, :], x2c, didx_sb[:], sz, sz, D,
                        transpose=False, single_packet=False,
                        queue_num=(gi + 1) % 4,
                    )
                    gi += 2
                    base += sz
                    for t in range(sz // MM_N):
                        tps = ptr_pool.tile([P, 2, MM_N], BF16, tag="tps")
                        xsT_ps = tps[:, 0, :]
                        xtT_ps = tps[:, 1, :]
                        for k in range(MM_N // P):
                            blk = t * (MM_N // P) + k
                            ksl = slice(k * P, (k + 1) * P)
                            nc.tensor.transpose(
                                xsT_ps[:, ksl], xs_rm[:, blk, :], id_sb[:]
                            )
                            nc.tensor.transpose(
                                xtT_ps[:, ksl], xt_rm[:, blk, :], id_sb[:]
                            )
                        xsT = tsb_pool.tile([P, MM_N], BF16, tag="xsT")
                        xtT = tsb_pool.tile([P, MM_N], BF16, tag="xtT")
                        nc.vector.tensor_copy(xsT[:], xsT_ps[:])
                        nc.scalar.activation(
                            xtT[:], xtT_ps[:], mybir.ActivationFunctionType.Copy
                        )
                        run_tile(xsT[:], xtT[:])
                # --- transposed slice (last tcall idxs of the bucket) ---
                # Issued after the bucket's NT gathers so the serialized
                # T chain never blocks the in-order Pool queue, and its
                # drain hides under the bucket's NT tile processing.
                # The spray xbar is a single serial stream: concurrent
                # transposed drains corrupt. The tidx pool (bufs=1) chains
                # each T gather behind the previous one's completion, so at
                # most one transposed gather is ever in flight, rotating
                # across rings for bandwidth balance.
                tco = tcall // 16
                tsidx = tidx_pool.tile([P, tco], I16, tag="t")
                nc.sync.dma_start(tsidx[:], sidx[b, :, ntc // 16 : ntc // 16 + tco])
                xsT_g = txs_pool.tile([P, 1, tcall], BF16, tag="xsTg")
                nc.gpsimd.dma_gather(
                    xsT_g[:], x1c, tsidx[:], tcall, tcall, D,
                    transpose=True, single_packet=False, queue_num=gi % 4,
                )
                gi += 1
                tdidx = tidx_pool.tile([P, tco], I16, tag="t")
                nc.sync.dma_start(tdidx[:], didx[b, :, ntc // 16 : ntc // 16 + tco])
                xtT_g = txt_pool.tile([P, 1, tcall], BF16, tag="xtTg")
                nc.gpsimd.dma_gather(
                    xtT_g[:], x2c, tdidx[:], tcall, tcall, D,
                    transpose=True, single_packet=False, queue_num=gi % 4,
                )
                gi += 1

                for t in range(tcall // MM_N):
                    ksl = slice(t * MM_N, (t + 1) * MM_N)
                    run_tile(xsT_g[:, 0, ksl], xtT_g[:, 0, ksl])
                drain_tiles()
            if orow:
                flush(orow)

    # The tile scheduler reorders instructions; SWDGE sem lanes are assigned
    # round-robin over the SCHEDULED order and each lane is locked to one
    # queue. Reassign queue_num = scheduled_index % 4 so lane l (index % 8)
    # always sees queue l % 4.
    def _walk(bb, idx=[0]):
        for inst in bb.instructions:
            if isinstance(inst, mybir.InstDMAGatherAnt):
                inst.queue_num = idx[0] % 4
                idx[0] += 1
            for b2 in getattr(inst, "bbs", []) or []:
                _walk(b2, idx)
    for bb in nc.main_func.blocks:
        _walk(bb)
    nc.compile()
    return nc


def _wrap_idx(arr, cap):
    """[NBUCKET, cap] int16 -> dma_gather layout [NBUCKET, 128, cap // 16]
    (idx i lives at partition i % 16, column i // 16; replicated 8x)."""
    w = arr.reshape(NBUCKET, cap // 16, 16).transpose(0, 2, 1)
    return np.tile(w, (1, 8, 1)).copy()


def _prep_core(src, dst, cap):
    """Bucket one core's edges by (src chunk, dst chunk). Returns the wrapped
    int16 local-index tensors (-1 padded), bucket-grouped edge order, counts."""
    bucket = (src // CHUNK) * NCHUNK + dst // CHUNK
    order = np.argsort(bucket, kind="stable")
    counts = np.bincount(bucket, minlength=NBUCKET)
    sloc = np.zeros(NBUCKET * cap, np.int16)
    dloc = np.zeros(NBUCKET * cap, np.int16)
    pos = 0
    for b in range(NBUCKET):
        grp = order[pos : pos + counts[b]]
        pos += counts[b]
        sloc[b * cap : b * cap + counts[b]] = src[grp] - (b // NCHUNK) * CHUNK
        dloc[b * cap : b * cap + counts[b]] = dst[grp] - (b % NCHUNK) * CHUNK
    return (
        _wrap_idx(sloc.reshape(NBUCKET, cap), cap),
        _wrap_idx(dloc.reshape(NBUCKET, cap), cap),
        order,
        counts,
    )


def kernel(x1, x2, edge_index, W1, b1, W2, b2, W3, b3, _trace=False):
    x1 = np.asarray(x1)
    x2 = np.asarray(x2)
    edge_index = np.asarray(edge_index)
    n_edges = edge_index.shape[1]
    assert x1.shape == (N_NODES, D) and x2.shape == (N_NODES, D)
    assert n_edges % N_CORES == 0
    e_core = n_edges // N_CORES

    x1bf = x1.astype(nbf)
    x2bf = x2.astype(nbf)
    W1 = np.asarray(W1, np.float32)
    w1a = W1[:D].astype(nbf)
    w1b = W1[D:].astype(nbf)
    w2 = np.asarray(W2, np.float32).astype(nbf)
    w3 = np.asarray(W3, np.float32).astype(nbf)
    b1c = np.asarray(b1, np.float32).reshape(D, 1)
    b2c = np.asarray(b2, np.float32).reshape(D, 1)
    b3c = np.full((P, 1), np.float32(np.asarray(b3).reshape(-1)[0]), np.float32)
    identc = np.eye(P, dtype=nbf)

    src_all = np.ascontiguousarray(edge_index[0]).astype(np.int64)
    dst_all = np.ascontiguousarray(edge_index[1]).astype(np.int64)

    preps = []
    max_count = 0
    for c in range(N_CORES):
        sl = slice(c * e_core, (c + 1) * e_core)
        src = src_all[sl]
        dst = dst_all[sl]
        counts = np.bincount((src // CHUNK) * NCHUNK + dst // CHUNK, minlength=NBUCKET)
        max_count = max(max_count, int(counts.max()))
        preps.append((src, dst))
    cap = CAP if max_count <= CAP else -(-max_count // (2 * MM_N)) * 2 * MM_N

    nc = build(cap)
    in_maps = []
    orders = []
    countss = []
    for src, dst in preps:
        sidx, didx, order, counts = _prep_core(src, dst, cap)
        orders.append(order)
        countss.append(counts)
        in_maps.append(
            {
                "x1bf": x1bf, "x2bf": x2bf, "sidx": sidx, "didx": didx,
                "w1a": w1a, "w1b": w1b, "w2": w2, "w3": w3,
                "b1": b1c, "b2": b2c, "b3": b3c, "ident": identc,
            }
        )

    res = run_bass_kernel_spmd(
        nc, in_maps, core_ids=list(range(N_CORES)), trace=_trace
    )
    kernel.last_result = res

    result = np.empty((n_edges,), np.float32)
    for c in range(N_CORES):
        flat = res.results[c]["out"].reshape(NBUCKET, cap)
        vals = np.concatenate(
            [flat[b, : countss[c][b]] for b in range(NBUCKET)]
        )
        r = np.empty((e_core,), np.float32)
        r[orders[c]] = vals
        result[c * e_core : (c + 1) * e_core] = r

    if _trace:
        kernel.last_exec_time_ns = res.exec_time_ns
    return result.reshape(n_edges, 1)


# revision 21
# speedup vs baseline: 1.3596x; 1.3596x over previous
"""GNN edge-MLP kernel for 8 TRN2 NeuronCores.

reference:
    xs = x1[edge_index[0]]; xt = x2[edge_index[1]]
    h = relu(concat(xs, xt) @ W1 + b1); h = relu(h @ W2 + b2); out = h @ W3 + b3

Strategy (pure edge parallelism, no collectives):
  - Edges sharded 8 ways (200k per core); node tables + weights replicated.
  - Host-side prep (numpy): cast tables/weights to bf16, bucket each core's
    edges by (src_chunk, dst_chunk) over 4 node chunks of 25000 rows so local
    gather indices fit in int16, pad buckets to a fixed capacity (pad idx=-1),
    pre-wrap indices into dma_gather's [16 x n/16] interleaved layout.
  - Device kernel: NON-transposed dma_gather (SWDGE) striped over all 4 SWDGE
    queues pulls edge rows row-major [128 edges x 128 feat] into SBUF.
    (Transposed gathers serialize on the shared spray xbar, so they cannot
    scale past ~1 ring; non-transposed gathers scale with the 4 rings.)
    The TensorEngine transposes each 128-edge block via identity matmul into
    PSUM (bf16), ACT/DVE copy the transposed tiles to SBUF, then the MLP:
      L1: psum = W1a.T @ xsT + W1b.T @ xtT     (two accumulating matmuls)
      s1 = relu(psum + b1)                      (DVE tensor_scalar add+max)
      L2: psum2 = W2.T @ s1 ; s2 = relu(psum2+b2)  (ACT activation)
      L3: obank[row] = W3.T @ s2                (M=1 matmul into a shared
                                                 psum bank, one row per tile)
    Output rows are flushed 32 tiles at a time (+ b3) to DRAM.
  - Host unpermutes bucket order back to the original edge order.
"""

import sys

sys.path.insert(0, "/opt/trn_rl_repo")

import functools

import ml_dtypes
import numpy as np

import concourse.bacc as bacc
import concourse.bass as bass
import concourse.mybir as mybir
import concourse.tile as tile
from concourse import library_config
from concourse.bass_utils import run_bass_kernel_spmd

P = 128
D = 128
N_NODES = 100000
N_EDGES = 1600000
N_CORES = 8
E_CORE = N_EDGES // N_CORES  # 200000
NCHUNK = 4
CHUNK = N_NODES // NCHUNK  # 25000 rows per node chunk (int16-safe)
NBUCKET = NCHUNK * NCHUNK  # 16
MM_N = 512  # matmul moving free dim (one PSUM f32 bank)
CAP = 13312  # per-bucket padded edge capacity (26 * 512); mean fill is 12500
TCALL = 3584  # idxs per bucket gathered TRANSPOSED (xbar path, serialized)
NTCALL = 3584  # max idxs per non-transposed gather call
FLUSH = 32  # tiles whose [1, 512] outputs accumulate in one psum bank

BF16 = mybir.dt.bfloat16
F32 = mybir.dt.float32
I16 = mybir.dt.int16
nbf = ml_dtypes.bfloat16


@functools.lru_cache(maxsize=2)
def build(cap=CAP):
    assert cap % MM_N == 0
    tcall = TCALL if cap >= 2 * TCALL else (cap // 2 // MM_N) * MM_N
    ntc = cap - tcall  # non-transposed idxs per bucket
    # 3 NT calls; sizes chosen so the 8 SWDGE instrs per bucket, assigned
    # queue = instr_index % 4, put equal descriptor volume on every ring.
    a = min(NTCALL, ntc - 2 * ((ntc // 3) // MM_N * MM_N))
    rem = ntc - a
    nt_sizes = [a, rem // 2, rem - rem // 2]
    assert all(sz > 0 and sz % MM_N == 0 for sz in nt_sizes), nt_sizes
    assert len(nt_sizes) == 3
    ntiles = NBUCKET * cap // MM_N
    nc = bacc.Bacc("TRN2", num_swdge_queues=4)
    x1bf = nc.dram_tensor("x1bf", [N_NODES, D], BF16, kind="ExternalInput")
    x2bf = nc.dram_tensor("x2bf", [N_NODES, D], BF16, kind="ExternalInput")
    sidx = nc.dram_tensor("sidx", [NBUCKET, P, cap // 16], I16, kind="ExternalInput")
    didx = nc.dram_tensor("didx", [NBUCKET, P, cap // 16], I16, kind="ExternalInput")
    w1a = nc.dram_tensor("w1a", [D, D], BF16, kind="ExternalInput")
    w1b = nc.dram_tensor("w1b", [D, D], BF16, kind="ExternalInput")
    w2 = nc.dram_tensor("w2", [D, D], BF16, kind="ExternalInput")
    w3 = nc.dram_tensor("w3", [D, 1], BF16, kind="ExternalInput")
    b1 = nc.dram_tensor("b1", [D, 1], F32, kind="ExternalInput")
    b2 = nc.dram_tensor("b2", [D, 1], F32, kind="ExternalInput")
    b3 = nc.dram_tensor("b3", [P, 1], F32, kind="ExternalInput")
    ident = nc.dram_tensor("ident", [P, P], BF16, kind="ExternalInput")
    out = nc.dram_tensor("out", [ntiles, MM_N], F32, kind="ExternalOutput")

    Relu = mybir.ActivationFunctionType.Relu
    Add = mybir.AluOpType.add
    Max = mybir.AluOpType.max

    with tile.TileContext(nc) as tc:
        nc.gpsimd.load_library(library_config.mlp)
        with (
            tc.tile_pool(name="const", bufs=1) as cpool,
            tc.tile_pool(name="gxs", bufs=3) as gxs_pool,
            tc.tile_pool(name="gxt", bufs=3) as gxt_pool,
            tc.tile_pool(name="txs", bufs=2) as txs_pool,
            tc.tile_pool(name="txt", bufs=2) as txt_pool,
            tc.tile_pool(name="idx", bufs=6) as idx_pool,
            tc.tile_pool(name="tidx", bufs=1) as tidx_pool,
            tc.tile_pool(name="tsb", bufs=6) as tsb_pool,
            tc.tile_pool(name="act", bufs=4) as act_pool,
            tc.tile_pool(name="osb", bufs=2) as out_pool,
            tc.tile_pool(name="ptr", bufs=2, space="PSUM") as ptr_pool,
            tc.tile_pool(name="ph", bufs=2, space="PSUM") as ph_pool,
            tc.tile_pool(name="pob", bufs=2, space="PSUM") as pob,
        ):
            w1a_sb = cpool.tile([D, D], BF16, tag="w1a")
            w1b_sb = cpool.tile([D, D], BF16, tag="w1b")
            w2_sb = cpool.tile([D, D], BF16, tag="w2")
            w3_sb = cpool.tile([D, 1], BF16, tag="w3")
            b1_sb = cpool.tile([D, 1], F32, tag="b1")
            b2_sb = cpool.tile([D, 1], F32, tag="b2")
            b3_sb = cpool.tile([P, 1], F32, tag="b3")
            id_sb = cpool.tile([P, P], BF16, tag="ident")
            for sb, dr in [
                (w1a_sb, w1a), (w1b_sb, w1b), (w2_sb, w2), (w3_sb, w3),
                (b1_sb, b1), (b2_sb, b2), (b3_sb, b3), (id_sb, ident),
            ]:
                nc.sync.dma_start(sb[:], dr[:])

            # w3 shifted into column m of slice m: L3's matmul for the m-th
            # tile of a flush group lands its [1, 512] result on psum
            # partition m (accumulating zeros onto every other row).
            w3m_sb = cpool.tile([P, FLUSH, D], BF16, tag="w3m")
            nc.vector.memset(w3m_sb[:], 0.0)
            for m in range(FLUSH):
                nc.vector.tensor_copy(w3m_sb[:, m, m : m + 1], w3_sb[:, 0:1])

            obank = None
            orow = 0
            oflushed = 0
            tile_no = 0

            def flush(rows):
                nonlocal obank, orow, oflushed
                osb = out_pool.tile([P, MM_N], F32, tag="osb")
                nc.vector.tensor_scalar_add(
                    osb[:rows, :], obank[:rows, :], b3_sb[:rows, 0:1]
                )
                nc.sync.dma_start(out[oflushed : oflushed + rows, :], osb[:rows, :])
                oflushed += rows
                obank = None
                orow = 0

            gi = 0  # SWDGE instruction counter; queue = gi % 4 always
            for b in range(NBUCKET):
                ci, cj = b // NCHUNK, b % NCHUNK
                x1c = x1bf[ci * CHUNK : (ci + 1) * CHUNK, :]
                x2c = x2bf[cj * CHUNK : (cj + 1) * CHUNK, :]

                # --- transposed slice (first tcall idxs of the bucket) ---
                # The spray xbar is a single serial stream: concurrent
                # transposed drains corrupt. The tidx pool (bufs=1) chains
                # each T gather behind the previous one's completion, so at
                # most one transposed gather is ever in flight, rotating
                # across rings for bandwidth balance.
                tco = tcall // 16
                tsidx = tidx_pool.tile([P, tco], I16, tag="t")
                nc.sync.dma_start(tsidx[:], sidx[b, :, 0:tco])
                xsT_g = txs_pool.tile([P, 1, tcall], BF16, tag="xsTg")
                nc.gpsimd.dma_gather(
                    xsT_g[:], x1c, tsidx[:], tcall, tcall, D,
                    transpose=True, single_packet=False, queue_num=gi % 4,
                )
                gi += 1
                tdidx = tidx_pool.tile([P, tco], I16, tag="t")
                nc.sync.dma_start(tdidx[:], didx[b, :, 0:tco])
                xtT_g = txt_pool.tile([P, 1, tcall], BF16, tag="xtTg")
                nc.gpsimd.dma_gather(
                    xtT_g[:], x2c, tdidx[:], tcall, tcall, D,
                    transpose=True, single_packet=False, queue_num=gi % 4,
                )
                gi += 1

                pend = []  # buffered tile inputs, flushed in pairs

                def run_pair(pair):
                    nonlocal obank, orow, tile_no
                    # Same-stationary matmuls issued back-to-back so the PE
                    # keeps each weight matrix loaded for both tiles.
                    h1s = []
                    for xsT_ap, xtT_ap in pair:
                        h1 = ph_pool.tile([P, MM_N], F32, tag="h1")
                        h1s.append(h1)
                        nc.tensor.matmul(
                            h1[:], w1a_sb[:], xsT_ap, start=True, stop=False
                        )
                    for h1, (xsT_ap, xtT_ap) in zip(h1s, pair):
                        nc.tensor.matmul(
                            h1[:], w1b_sb[:], xtT_ap, start=False, stop=True
                        )
                    s1s = []
                    for h1 in h1s:
                        s1 = act_pool.tile([P, MM_N], BF16, tag="s1")
                        s1s.append(s1)
                        nc.vector.tensor_scalar(
                            s1[:], h1[:], b1_sb[:, 0:1], 0.0, Add, Max
                        )
                    h2s = []
                    for s1 in s1s:
                        h2 = ph_pool.tile([P, MM_N], F32, tag="h2")
                        h2s.append(h2)
                        nc.tensor.matmul(h2[:], w2_sb[:], s1[:], start=True, stop=True)
                    s2s = []
                    for h2 in h2s:
                        s2 = act_pool.tile([P, MM_N], BF16, tag="s2")
                        s2s.append(s2)
                        nc.scalar.activation(s2[:], h2[:], Relu, bias=b2_sb[:, 0:1])
                    for s2 in s2s:
                        if obank is None:
                            obank = pob.tile([P, MM_N], F32, tag="ob")
                        grp = min(FLUSH, ntiles - (tile_no - orow))
                        nc.tensor.matmul(
                            obank[:], w3m_sb[:, orow, :], s2[:],
                            start=(orow == 0), stop=(orow == grp - 1),
                        )
                        orow += 1
                        tile_no += 1
                        if orow == grp:
                            flush(grp)

                def run_tile(xsT_ap, xtT_ap, force=False):
                    pend.append((xsT_ap, xtT_ap))
                    if len(pend) == 2:
                        run_pair(pend[:])
                        pend.clear()

                def drain_tiles():
                    if pend:
                        run_pair(pend[:])
                        pend.clear()

                for t in range(tcall // MM_N):
                    ksl = slice(t * MM_N, (t + 1) * MM_N)
                    run_tile(xsT_g[:, 0, ksl], xtT_g[:, 0, ksl])

                # --- non-transposed slices (rest of the bucket) ---
                base = tcall
                for sz in nt_sizes:
                    nblk = sz // P
                    c0 = base // 16
                    cols = sz // 16
                    sidx_sb = idx_pool.tile([P, cols], I16, tag="sidx")
                    didx_sb = idx_pool.tile([P, cols], I16, tag="didx")
                    nc.sync.dma_start(sidx_sb[:], sidx[b, :, c0 : c0 + cols])
                    nc.sync.dma_start(didx_sb[:], didx[b, :, c0 : c0 + cols])
                    xs_rm = gxs_pool.tile([P, NTCALL // P, D], BF16, tag="xs")
                    xt_rm = gxt_pool.tile([P, NTCALL // P, D], BF16, tag="xt")
                    nc.gpsimd.dma_gather(
                        xs_rm[:, :nblk, :], x1c, sidx_sb[:], sz, sz, D,
                        transpose=False, single_packet=False,
                        queue_num=gi % 4,
                    )
                    nc.gpsimd.dma_gather(
                        xt_rm[:, :nblk, :], x2c, didx_sb[:], sz, sz, D,
                        transpose=False, single_packet=False,
                        queue_num=(gi + 1) % 4,
                    )
                    gi += 2
                    base += sz
                    for t in range(sz // MM_N):
                        tps = ptr_pool.tile([P, 2, MM_N], BF16, tag="tps")
                        xsT_ps = tps[:, 0, :]
                        xtT_ps = tps[:, 1, :]
                        for k in range(MM_N // P):
                            blk = t * (MM_N // P) + k
                            ksl = slice(k * P, (k + 1) * P)
                            nc.tensor.transpose(
                                xsT_ps[:, ksl], xs_rm[:, blk, :], id_sb[:]
                            )
                            nc.tensor.transpose(
                                xtT_ps[:, ksl], xt_rm[:, blk, :], id_sb[:]
                            )
                        xsT = tsb_pool.tile([P, MM_N], BF16, tag="xsT")
                        xtT = tsb_pool.tile([P, MM_N], BF16, tag="xtT")
                        nc.vector.tensor_copy(xsT[:], xsT_ps[:])
                        nc.scalar.activation(
                            xtT[:], xtT_ps[:], mybir.ActivationFunctionType.Copy
                        )
                        run_tile(xsT[:], xtT[:])
                drain_tiles()
            if orow:
                flush(orow)

    # The tile scheduler reorders instructions; SWDGE sem lanes are assigned
    # round-robin over the SCHEDULED order and each lane is locked to one
    # queue. Reassign queue_num = scheduled_index % 4 so lane l (index % 8)
    # always sees queue l % 4.
    def _walk(bb, idx=[0]):
        for inst in bb.instructions:
            if isinstance(inst, mybir.InstDMAGatherAnt):
                inst.queue_num = idx[0] % 4
                idx[0] += 1
            for b2 in getattr(inst, "bbs", []) or []:
                _walk(b2, idx)
    for bb in nc.main_func.blocks:
        _walk(bb)
    nc.compile()
    return nc


def _wrap_idx(arr, cap):
    """[NBUCKET, cap] int16 -> dma_gather layout [NBUCKET, 128, cap // 16]
    (idx i lives at partition i % 16, column i // 16; replicated 8x)."""
    w = arr.reshape(NBUCKET, cap // 16, 16).transpose(0, 2, 1)
    return np.tile(w, (1, 8, 1)).copy()


def _prep_core(src, dst, cap):
    """Bucket one core's edges by (src chunk, dst chunk). Returns the wrapped
    int16 local-index tensors (-1 padded), bucket-grouped edge order, counts."""
    bucket = (src // CHUNK) * NCHUNK + dst // CHUNK
    order = np.argsort(bucket, kind="stable")
    counts = np.bincount(bucket, minlength=NBUCKET)
    sloc = np.zeros(NBUCKET * cap, np.int16)
    dloc = np.zeros(NBUCKET * cap, np.int16)
    pos = 0
    for b in range(NBUCKET):
        grp = order[pos : pos + counts[b]]
        pos += counts[b]
        sloc[b * cap : b * cap + counts[b]] = src[grp] - (b // NCHUNK) * CHUNK
        dloc[b * cap : b * cap + counts[b]] = dst[grp] - (b % NCHUNK) * CHUNK
    return (
        _wrap_idx(sloc.reshape(NBUCKET, cap), cap),
        _wrap_idx(dloc.reshape(NBUCKET, cap), cap),
        order,
        counts,
    )


def kernel(x1, x2, edge_index, W1, b1, W2, b2, W3, b3, _trace=False):
    x1 = np.asarray(x1)
    x2 = np.asarray(x2)
    edge_index = np.asarray(edge_index)
    n_edges = edge_index.shape[1]
    assert x1.shape == (N_NODES, D) and x2.shape == (N_NODES, D)
    assert n_edges % N_CORES == 0
    e_core = n_edges // N_CORES

    x1bf = x1.astype(nbf)
    x2bf = x2.astype(nbf)
    W1 = np.asarray(W1, np.float32)
    w1a = W1[:D].astype(nbf)
    w1b = W1[D:].astype(nbf)
    w2 = np.asarray(W2, np.float32).astype(nbf)
    w3 = np.asarray(W3, np.float32).astype(nbf)
    b1c = np.asarray(b1, np.float32).reshape(D, 1)
    b2c = np.asarray(b2, np.float32).reshape(D, 1)
    b3c = np.full((P, 1), np.float32(np.asarray(b3).reshape(-1)[0]), np.float32)
    identc = np.eye(P, dtype=nbf)

    src_all = np.ascontiguousarray(edge_index[0]).astype(np.int64)
    dst_all = np.ascontiguousarray(edge_index[1]).astype(np.int64)

    preps = []
    max_count = 0
    for c in range(N_CORES):
        sl = slice(c * e_core, (c + 1) * e_core)
        src = src_all[sl]
        dst = dst_all[sl]
        counts = np.bincount((src // CHUNK) * NCHUNK + dst // CHUNK, minlength=NBUCKET)
        max_count = max(max_count, int(counts.max()))
        preps.append((src, dst))
    cap = CAP if max_count <= CAP else -(-max_count // (2 * MM_N)) * 2 * MM_N

    nc = build(cap)
    in_maps = []
    orders = []
    countss = []
    for src, dst in preps:
        sidx, didx, order, counts = _prep_core(src, dst, cap)
        orders.append(order)
        countss.append(counts)
        in_maps.append(
            {
                "x1bf": x1bf, "x2bf": x2bf, "sidx": sidx, "didx": didx,
                "w1a": w1a, "w1b": w1b, "w2": w2, "w3": w3,
                "b1": b1c, "b2": b2c, "b3": b3c, "ident": identc,
            }
        )

    res = run_bass_kernel_spmd(
        nc, in_maps, core_ids=list(range(N_CORES)), trace=_trace
    )
    kernel.last_result = res

    result = np.empty((n_edges,), np.float32)
    for c in range(N_CORES):
        flat = res.results[c]["out"].reshape(NBUCKET, cap)
        vals = np.concatenate(
            [flat[b, : countss[c][b]] for b in range(NBUCKET)]
        )
        r = np.empty((e_core,), np.float32)
        r[orders[c]] = vals
        result[c * e_core : (c + 1) * e_core] = r

    if _trace:
        kernel.last_exec_time_ns = res.exec_time_ns
    return result.reshape(n_edges, 1)


# revision 22
# speedup vs baseline: 1.4804x; 1.0888x over previous
"""GNN edge-MLP kernel for 8 TRN2 NeuronCores.

reference:
    xs = x1[edge_index[0]]; xt = x2[edge_index[1]]
    h = relu(concat(xs, xt) @ W1 + b1); h = relu(h @ W2 + b2); out = h @ W3 + b3

Strategy (pure edge parallelism, no collectives):
  - Edges sharded 8 ways (200k per core); node tables + weights replicated.
  - Host-side prep (numpy): cast tables/weights to bf16, bucket each core's
    edges by (src_chunk, dst_chunk) over 4 node chunks of 25000 rows so local
    gather indices fit in int16, pad buckets to a fixed capacity (pad idx=-1),
    pre-wrap indices into dma_gather's [16 x n/16] interleaved layout.
  - Device kernel: NON-transposed dma_gather (SWDGE) striped over all 4 SWDGE
    queues pulls edge rows row-major [128 edges x 128 feat] into SBUF.
    (Transposed gathers serialize on the shared spray xbar, so they cannot
    scale past ~1 ring; non-transposed gathers scale with the 4 rings.)
    The TensorEngine transposes each 128-edge block via identity matmul into
    PSUM (bf16), ACT/DVE copy the transposed tiles to SBUF, then the MLP:
      L1: psum = W1a.T @ xsT + W1b.T @ xtT     (two accumulating matmuls)
      s1 = relu(psum + b1)                      (DVE tensor_scalar add+max)
      L2: psum2 = W2.T @ s1 ; s2 = relu(psum2+b2)  (ACT activation)
      L3: obank[row] = W3.T @ s2                (M=1 matmul into a shared
                                                 psum bank, one row per tile)
    Output rows are flushed 32 tiles at a time (+ b3) to DRAM.
  - Host unpermutes bucket order back to the original edge order.
"""

import sys

sys.path.insert(0, "/opt/trn_rl_repo")

import functools

import ml_dtypes
import numpy as np

import concourse.bacc as bacc
import concourse.bass as bass
import concourse.mybir as mybir
import concourse.tile as tile
from concourse import library_config
from concourse.bass_utils import run_bass_kernel_spmd

P = 128
D = 128
N_NODES = 100000
N_EDGES = 1600000
N_CORES = 8
E_CORE = N_EDGES // N_CORES  # 200000
NCHUNK = 4
CHUNK = N_NODES // NCHUNK  # 25000 rows per node chunk (int16-safe)
NBUCKET = NCHUNK * NCHUNK  # 16
MM_N = 512  # matmul moving free dim (one PSUM f32 bank)
CAP = 13312  # per-bucket padded edge capacity (26 * 512); mean fill is 12500
TCALL = 3584  # idxs per bucket gathered TRANSPOSED (xbar path, serialized)
NTCALL = 3584  # max idxs per non-transposed gather call
FLUSH = 32  # tiles whose [1, 512] outputs accumulate in one psum bank

BF16 = mybir.dt.bfloat16
F32 = mybir.dt.float32
I16 = mybir.dt.int16
nbf = ml_dtypes.bfloat16


@functools.lru_cache(maxsize=2)
def build(cap=CAP):
    assert cap % MM_N == 0
    tcall = TCALL if cap >= 2 * TCALL else (cap // 2 // MM_N) * MM_N
    ntc = cap - tcall  # non-transposed idxs per bucket
    # 3 NT calls; sizes chosen so the 8 SWDGE instrs per bucket, assigned
    # queue = instr_index % 4, put equal descriptor volume on every ring.
    a = min(NTCALL, ntc - 2 * ((ntc // 3) // MM_N * MM_N))
    rem = ntc - a
    nt_sizes = [a, rem // 2, rem - rem // 2]
    assert all(sz > 0 and sz % MM_N == 0 for sz in nt_sizes), nt_sizes
    assert len(nt_sizes) == 3
    ntiles = NBUCKET * cap // MM_N
    nc = bacc.Bacc("TRN2", num_swdge_queues=4)
    x1bf = nc.dram_tensor("x1bf", [N_NODES, D], BF16, kind="ExternalInput")
    x2bf = nc.dram_tensor("x2bf", [N_NODES, D], BF16, kind="ExternalInput")
    sidx = nc.dram_tensor("sidx", [NBUCKET, P, cap // 16], I16, kind="ExternalInput")
    didx = nc.dram_tensor("didx", [NBUCKET, P, cap // 16], I16, kind="ExternalInput")
    w1a = nc.dram_tensor("w1a", [D, D], BF16, kind="ExternalInput")
    w1b = nc.dram_tensor("w1b", [D, D], BF16, kind="ExternalInput")
    w2 = nc.dram_tensor("w2", [D, D], BF16, kind="ExternalInput")
    w3 = nc.dram_tensor("w3", [D, 1], BF16, kind="ExternalInput")
    b1 = nc.dram_tensor("b1", [D, 1], F32, kind="ExternalInput")
    b2 = nc.dram_tensor("b2", [D, 1], F32, kind="ExternalInput")
    b3 = nc.dram_tensor("b3", [P, 1], F32, kind="ExternalInput")
    ident = nc.dram_tensor("ident", [P, P], BF16, kind="ExternalInput")
    out = nc.dram_tensor("out", [ntiles, MM_N], F32, kind="ExternalOutput")

    Relu = mybir.ActivationFunctionType.Relu
    Add = mybir.AluOpType.add
    Max = mybir.AluOpType.max

    with tile.TileContext(nc) as tc:
        nc.gpsimd.load_library(library_config.mlp)
        with (
            tc.tile_pool(name="const", bufs=1) as cpool,
            tc.tile_pool(name="gxs", bufs=4) as gxs_pool,
            tc.tile_pool(name="gxt", bufs=4) as gxt_pool,
            tc.tile_pool(name="txs", bufs=3) as txs_pool,
            tc.tile_pool(name="txt", bufs=3) as txt_pool,
            tc.tile_pool(name="idx", bufs=8) as idx_pool,
            tc.tile_pool(name="tidx", bufs=1) as tidx_pool,
            tc.tile_pool(name="tsb", bufs=6) as tsb_pool,
            tc.tile_pool(name="act", bufs=4) as act_pool,
            tc.tile_pool(name="osb", bufs=2) as out_pool,
            tc.tile_pool(name="ptr", bufs=2, space="PSUM") as ptr_pool,
            tc.tile_pool(name="ph", bufs=2, space="PSUM") as ph_pool,
            tc.tile_pool(name="pob", bufs=2, space="PSUM") as pob,
        ):
            w1a_sb = cpool.tile([D, D], BF16, tag="w1a")
            w1b_sb = cpool.tile([D, D], BF16, tag="w1b")
            w2_sb = cpool.tile([D, D], BF16, tag="w2")
            w3_sb = cpool.tile([D, 1], BF16, tag="w3")
            b1_sb = cpool.tile([D, 1], F32, tag="b1")
            b2_sb = cpool.tile([D, 1], F32, tag="b2")
            b3_sb = cpool.tile([P, 1], F32, tag="b3")
            id_sb = cpool.tile([P, P], BF16, tag="ident")
            for sb, dr in [
                (w1a_sb, w1a), (w1b_sb, w1b), (w2_sb, w2), (w3_sb, w3),
                (b1_sb, b1), (b2_sb, b2), (b3_sb, b3), (id_sb, ident),
            ]:
                nc.sync.dma_start(sb[:], dr[:])

            # w3 shifted into column m of slice m: L3's matmul for the m-th
            # tile of a flush group lands its [1, 512] result on psum
            # partition m (accumulating zeros onto every other row).
            w3m_sb = cpool.tile([P, FLUSH, D], BF16, tag="w3m")
            nc.vector.memset(w3m_sb[:], 0.0)
            for m in range(FLUSH):
                nc.vector.tensor_copy(w3m_sb[:, m, m : m + 1], w3_sb[:, 0:1])

            obank = None
            orow = 0
            oflushed = 0
            tile_no = 0

            def flush(rows):
                nonlocal obank, orow, oflushed
                osb = out_pool.tile([P, MM_N], F32, tag="osb")
                nc.vector.tensor_scalar_add(
                    osb[:rows, :], obank[:rows, :], b3_sb[:rows, 0:1]
                )
                nc.sync.dma_start(out[oflushed : oflushed + rows, :], osb[:rows, :])
                oflushed += rows
                obank = None
                orow = 0

            gi = 0  # SWDGE instruction counter; queue = gi % 4 always
            for b in range(NBUCKET):
                ci, cj = b // NCHUNK, b % NCHUNK
                x1c = x1bf[ci * CHUNK : (ci + 1) * CHUNK, :]
                x2c = x2bf[cj * CHUNK : (cj + 1) * CHUNK, :]

                # --- transposed slice (first tcall idxs of the bucket) ---
                # The spray xbar is a single serial stream: concurrent
                # transposed drains corrupt. The tidx pool (bufs=1) chains
                # each T gather behind the previous one's completion, so at
                # most one transposed gather is ever in flight, rotating
                # across rings for bandwidth balance.
                tco = tcall // 16
                tsidx = tidx_pool.tile([P, tco], I16, tag="t")
                nc.sync.dma_start(tsidx[:], sidx[b, :, 0:tco])
                xsT_g = txs_pool.tile([P, 1, tcall], BF16, tag="xsTg")
                nc.gpsimd.dma_gather(
                    xsT_g[:], x1c, tsidx[:], tcall, tcall, D,
                    transpose=True, single_packet=False, queue_num=gi % 4,
                )
                gi += 1
                tdidx = tidx_pool.tile([P, tco], I16, tag="t")
                nc.sync.dma_start(tdidx[:], didx[b, :, 0:tco])
                xtT_g = txt_pool.tile([P, 1, tcall], BF16, tag="xtTg")
                nc.gpsimd.dma_gather(
                    xtT_g[:], x2c, tdidx[:], tcall, tcall, D,
                    transpose=True, single_packet=False, queue_num=gi % 4,
                )
                gi += 1

                pend = []  # buffered tile inputs, flushed in pairs

                def run_pair(pair):
                    nonlocal obank, orow, tile_no
                    # Same-stationary matmuls issued back-to-back so the PE
                    # keeps each weight matrix loaded for both tiles.
                    h1s = []
                    for xsT_ap, xtT_ap in pair:
                        h1 = ph_pool.tile([P, MM_N], F32, tag="h1")
                        h1s.append(h1)
                        nc.tensor.matmul(
                            h1[:], w1a_sb[:], xsT_ap, start=True, stop=False
                        )
                    for h1, (xsT_ap, xtT_ap) in zip(h1s, pair):
                        nc.tensor.matmul(
                            h1[:], w1b_sb[:], xtT_ap, start=False, stop=True
                        )
                    s1s = []
                    for h1 in h1s:
                        s1 = act_pool.tile([P, MM_N], BF16, tag="s1")
                        s1s.append(s1)
                        nc.vector.tensor_scalar(
                            s1[:], h1[:], b1_sb[:, 0:1], 0.0, Add, Max
                        )
                    h2s = []
                    for s1 in s1s:
                        h2 = ph_pool.tile([P, MM_N], F32, tag="h2")
                        h2s.append(h2)
                        nc.tensor.matmul(h2[:], w2_sb[:], s1[:], start=True, stop=True)
                    s2s = []
                    for h2 in h2s:
                        s2 = act_pool.tile([P, MM_N], BF16, tag="s2")
                        s2s.append(s2)
                        nc.scalar.activation(s2[:], h2[:], Relu, bias=b2_sb[:, 0:1])
                    for s2 in s2s:
                        if obank is None:
                            obank = pob.tile([P, MM_N], F32, tag="ob")
                        grp = min(FLUSH, ntiles - (tile_no - orow))
                        nc.tensor.matmul(
                            obank[:], w3m_sb[:, orow, :], s2[:],
                            start=(orow == 0), stop=(orow == grp - 1),
                        )
                        orow += 1
                        tile_no += 1
                        if orow == grp:
                            flush(grp)

                def run_tile(xsT_ap, xtT_ap, force=False):
                    pend.append((xsT_ap, xtT_ap))
                    if len(pend) == 2:
                        run_pair(pend[:])
                        pend.clear()

                def drain_tiles():
                    if pend:
                        run_pair(pend[:])
                        pend.clear()

                for t in range(tcall // MM_N):
                    ksl = slice(t * MM_N, (t + 1) * MM_N)
                    run_tile(xsT_g[:, 0, ksl], xtT_g[:, 0, ksl])

                # --- non-transposed slices (rest of the bucket) ---
                base = tcall
                for sz in nt_sizes:
                    nblk = sz // P
                    c0 = base // 16
                    cols = sz // 16
                    sidx_sb = idx_pool.tile([P, cols], I16, tag="sidx")
                    didx_sb = idx_pool.tile([P, cols], I16, tag="didx")
                    nc.sync.dma_start(sidx_sb[:], sidx[b, :, c0 : c0 + cols])
                    nc.sync.dma_start(didx_sb[:], didx[b, :, c0 : c0 + cols])
                    xs_rm = gxs_pool.tile([P, NTCALL // P, D], BF16, tag="xs")
                    xt_rm = gxt_pool.tile([P, NTCALL // P, D], BF16, tag="xt")
                    nc.gpsimd.dma_gather(
                        xs_rm[:, :nblk, :], x1c, sidx_sb[:], sz, sz, D,
                        transpose=False, single_packet=False,
                        queue_num=gi % 4,
                    )
                    nc.gpsimd.dma_gather(
                        xt_rm[:, :nblk, :], x2c, didx_sb[:], sz, sz, D,
                        transpose=False, single_packet=False,
                        queue_num=(gi + 1) % 4,
                    )
                    gi += 2
                    base += sz
                    for t in range(sz // MM_N):
                        tps = ptr_pool.tile([P, 2, MM_N], BF16, tag="tps")
                        xsT_ps = tps[:, 0, :]
                        xtT_ps = tps[:, 1, :]
                        for k in range(MM_N // P):
                            blk = t * (MM_N // P) + k
                            ksl = slice(k * P, (k + 1) * P)
                            nc.tensor.transpose(
                                xsT_ps[:, ksl], xs_rm[:, blk, :], id_sb[:]
                            )
                            nc.tensor.transpose(
                                xtT_ps[:, ksl], xt_rm[:, blk, :], id_sb[:]
                            )
                        xsT = tsb_pool.tile([P, MM_N], BF16, tag="xsT")
                        xtT = tsb_pool.tile([P, MM_N], BF16, tag="xtT")
                        nc.vector.tensor_copy(xsT[:], xsT_ps[:])
                        nc.scalar.activation(
                            xtT[:], xtT_ps[:], mybir.ActivationFunctionType.Copy
                        )
                        run_tile(xsT[:], xtT[:])
                drain_tiles()
            if orow:
                flush(orow)

    # The tile scheduler reorders instructions; SWDGE sem lanes are assigned
    # round-robin over the SCHEDULED order and each lane is locked to one
    # queue. Reassign queue_num = scheduled_index % 4 so lane l (index % 8)
    # always sees queue l % 4.
    def _walk(bb, idx=[0]):
        for inst in bb.instructions:
            if isinstance(inst, mybir.InstDMAGatherAnt):
                inst.queue_num = idx[0] % 4
                idx[0] += 1
            for b2 in getattr(inst, "bbs", []) or []:
                _walk(b2, idx)
    for bb in nc.main_func.blocks:
        _walk(bb)
    nc.compile()
    return nc


def _wrap_idx(arr, cap):
    """[NBUCKET, cap] int16 -> dma_gather layout [NBUCKET, 128, cap // 16]
    (idx i lives at partition i % 16, column i // 16; replicated 8x)."""
    w = arr.reshape(NBUCKET, cap // 16, 16).transpose(0, 2, 1)
    return np.tile(w, (1, 8, 1)).copy()


def _prep_core(src, dst, cap):
    """Bucket one core's edges by (src chunk, dst chunk). Returns the wrapped
    int16 local-index tensors (-1 padded), bucket-grouped edge order, counts."""
    bucket = (src // CHUNK) * NCHUNK + dst // CHUNK
    order = np.argsort(bucket, kind="stable")
    counts = np.bincount(bucket, minlength=NBUCKET)
    sloc = np.zeros(NBUCKET * cap, np.int16)
    dloc = np.zeros(NBUCKET * cap, np.int16)
    pos = 0
    for b in range(NBUCKET):
        grp = order[pos : pos + counts[b]]
        pos += counts[b]
        sloc[b * cap : b * cap + counts[b]] = src[grp] - (b // NCHUNK) * CHUNK
        dloc[b * cap : b * cap + counts[b]] = dst[grp] - (b % NCHUNK) * CHUNK
    return (
        _wrap_idx(sloc.reshape(NBUCKET, cap), cap),
        _wrap_idx(dloc.reshape(NBUCKET, cap), cap),
        order,
        counts,
    )


def kernel(x1, x2, edge_index, W1, b1, W2, b2, W3, b3, _trace=False):
    x1 = np.asarray(x1)
    x2 = np.asarray(x2)
    edge_index = np.asarray(edge_index)
    n_edges = edge_index.shape[1]
    assert x1.shape == (N_NODES, D) and x2.shape == (N_NODES, D)
    assert n_edges % N_CORES == 0
    e_core = n_edges // N_CORES

    x1bf = x1.astype(nbf)
    x2bf = x2.astype(nbf)
    W1 = np.asarray(W1, np.float32)
    w1a = W1[:D].astype(nbf)
    w1b = W1[D:].astype(nbf)
    w2 = np.asarray(W2, np.float32).astype(nbf)
    w3 = np.asarray(W3, np.float32).astype(nbf)
    b1c = np.asarray(b1, np.float32).reshape(D, 1)
    b2c = np.asarray(b2, np.float32).reshape(D, 1)
    b3c = np.full((P, 1), np.float32(np.asarray(b3).reshape(-1)[0]), np.float32)
    identc = np.eye(P, dtype=nbf)

    src_all = np.ascontiguousarray(edge_index[0]).astype(np.int64)
    dst_all = np.ascontiguousarray(edge_index[1]).astype(np.int64)

    preps = []
    max_count = 0
    for c in range(N_CORES):
        sl = slice(c * e_core, (c + 1) * e_core)
        src = src_all[sl]
        dst = dst_all[sl]
        counts = np.bincount((src // CHUNK) * NCHUNK + dst // CHUNK, minlength=NBUCKET)
        max_count = max(max_count, int(counts.max()))
        preps.append((src, dst))
    cap = CAP if max_count <= CAP else -(-max_count // (2 * MM_N)) * 2 * MM_N

    nc = build(cap)
    in_maps = []
    orders = []
    countss = []
    for src, dst in preps:
        sidx, didx, order, counts = _prep_core(src, dst, cap)
        orders.append(order)
        countss.append(counts)
        in_maps.append(
            {
                "x1bf": x1bf, "x2bf": x2bf, "sidx": sidx, "didx": didx,
                "w1a": w1a, "w1b": w1b, "w2": w2, "w3": w3,
                "b1": b1c, "b2": b2c, "b3": b3c, "ident": identc,
            }
        )

    res = run_bass_kernel_spmd(
        nc, in_maps, core_ids=list(range(N_CORES)), trace=_trace
    )
    kernel.last_result = res

    result = np.empty((n_edges,), np.float32)
    for c in range(N_CORES):
        flat = res.results[c]["out"].reshape(NBUCKET, cap)
        vals = np.concatenate(
            [flat[b, : countss[c][b]] for b in range(NBUCKET)]
        )
        r = np.empty((e_core,), np.float32)
        r[orders[c]] = vals
        result[c * e_core : (c + 1) * e_core] = r

    if _trace:
        kernel.last_exec_time_ns = res.exec_time_ns
    return result.reshape(n_edges, 1)
